# revision 4
# baseline (speedup 1.0000x reference)
"""Trainium2 Bass kernel for nn_DFE_model (gnn_message_passing).

Math: reference scatters upd[m,i] = A_vals[i]*X[m, A_cols[i]//2] -
V[A_rows[i], A_cols[i]] into D[m, :, :] (last write wins per (row, col)),
then H[m] = sum_j F[j] * exp(-sum_k W[j,k]*relu(D[m,j,k])^2).

Per active slot s (j, k, f=k//2) with P = sqrt(W)*a, Q = sqrt(W)*V, the
contribution to E[j, m] is relu(P*x[m,f] - Q)^2.  Layout: X^T stays
resident in SBUF as three [128, 512] feature tiles (identity + one spill
tile per sign); each batch of 128 slots (a "round") is one per-partition
affine+relu over a resident tile followed by a relu^2 product and a
[128 slot -> 64 j] fp16 mask matmul accumulating E[64, 512] in PSUM.
P^2 is folded into the mask weights, so the per-round ops are:
  u  = max(x + (-t), 0)   (P>0 rounds; min for P<0 -- square kills sign)
  r2 = (x + (-t)) * u     (scalar_tensor_tensor == relu(x-t)^2 exactly)
both InstTensorScalarPtr (fast DVE mode), or ACT Relu with per-partition
scale/bias for rounds assigned to the scalar engine.

Host classifies slots exactly (per-feature min/max of the actual X):
sure-zero dropped; wild always-on slots (|t| > 100 or P == 0) use one
ACT Square(scale=P, bias=-Q) round (no relu needed, mask 1.0); the rest
pack into sign-homogeneous rounds on 3 resident tiles.  E is copied
PSUM->SBUF and DMAed out per core; the host does exp and the F-weighted
reduction (cheap: 8 x [64, 512]).
"""

import numpy as np

import concourse.bass as bass
import concourse.mybir as mybir
import concourse.tile as tile
from concourse.bass_utils import run_bass_kernel_spmd

# ---------------------------------------------------------------- constants
M = 512
J = 512
K = 256
NF = 128
NCORES = 8
JC = J // NCORES

_DT = mybir.dt.float32
_DT16 = mybir.dt.float16
_NP16 = np.float16

WILD_T = 100.0

OP_MAX, OP_MIN, OP_FREE = 0, 1, 2


# ------------------------------------------------------- walrus wait limit
def _legalize_waits(nc, max_waits=1):
    n = 0
    for f in nc.m.functions:
        for b in f.blocks:
            out, changed = [], False
            for inst in list(b.instructions):
                si = inst.sync_info
                waits = list(si.on_wait) if si and si.on_wait else []
                if len(waits) > max_waits:
                    for w in waits[max_waits:]:
                        n += 1
                        nop = mybir.InstNoOp(name=f"waitfix_{n}", ins=[], outs=[])
                        nop.engine = inst.engine
                        nop.sync_info = mybir.SyncInfo(on_wait=[w], on_update=[])
                        out.append(nop)
                    si.on_wait = waits[:max_waits]
                    changed = True
                out.append(inst)
            if changed:
                b.instructions = out


# ------------------------------------------------ slim Tile exit barrier
def _slim_drain_and_barrier(self, tick_clock, wait_clock):
    from concourse.vector_clock import ScopedClock

    drain_sp = self.nc.sync.drain()
    wait_clock.add_sem_waits(
        drain_sp.ins, ScopedClock({None: tick_clock.global_clock})
    )
    drain_gp = self.nc.gpsimd.drain()
    wait_clock.add_sem_waits(
        drain_gp.ins, ScopedClock({None: tick_clock.global_clock})
    )
    assert self.sems is not None
    popped = self.nc._tile_sem_poison_stack.pop()
    assert popped is self._sem_poison
    self.nc.clear_and_free_semaphores(list(self.sems.allocated().values()))


tile.TileContext._drain_and_barrier = _slim_drain_and_barrier


# ---------------------------------------------------------------- packing
def _prepare(X, A_vals, V, W, Fvec, A_rows, A_cols):
    rows = np.asarray(A_rows).astype(np.int64)
    cols = np.asarray(A_cols).astype(np.int64)
    X = np.asarray(X, dtype=np.float32)
    A_vals = np.asarray(A_vals, dtype=np.float32)
    V = np.asarray(V, dtype=np.float32)
    W = np.asarray(W, dtype=np.float32)
    Fvec = np.asarray(Fvec, dtype=np.float32)

    nnz = rows.shape[0]
    lin = rows * K + cols
    winner = np.full(J * K, -1, dtype=np.int64)
    winner[lin] = np.arange(nnz)
    active = np.nonzero(winner >= 0)[0]
    i = winner[active]
    j = active // K
    k = active % K
    f = k // 2
    s = np.sqrt(W[j, k]).astype(np.float32)
    P = s * A_vals[i]
    Q = s * V[j, k]

    xmin = X.min(axis=0)
    xmax = X.max(axis=0)
    with np.errstate(divide="ignore", invalid="ignore"):
        t = np.where(P != 0, Q / np.where(P == 0, 1.0, P), 0.0)
    pos = P > 0
    neg = P < 0
    zer = P == 0
    sure_zero = (
        (pos & (t >= xmax[f])) | (neg & (t <= xmin[f])) | (zer & (Q >= 0))
    )
    sure_on = (
        (pos & (t <= xmin[f])) | (neg & (t >= xmax[f])) | (zer & (Q < 0))
    )
    keep = ~sure_zero
    wild = keep & sure_on & (zer | (np.abs(t) > WILD_T))
    tame = keep & ~wild

    core = j // JC
    jl = j % JC

    npos = np.zeros((NCORES, NF), np.int64)
    nneg = np.zeros((NCORES, NF), np.int64)
    nwld = np.zeros((NCORES, NF), np.int64)
    for c in range(NCORES):
        cs = core == c
        npos[c] = np.bincount(f[cs & tame & pos], minlength=NF)
        nneg[c] = np.bincount(f[cs & tame & neg], minlength=NF)
        nwld[c] = np.bincount(f[cs & wild], minlength=NF)

    def spill_ok(n_cf, RI, RS):
        ov = np.maximum(0, n_cf - RI)
        if RS == 0:
            return not np.any(ov > 0)
        return np.ceil(ov / RS).sum() <= NF

    def search(n_all):
        best = None
        for RI in range(0, 20):
            for RS in range(0, 10):
                if best is not None and RI + RS >= best[0] + best[1]:
                    continue
                if all(spill_ok(n_all[c], RI, RS) for c in range(NCORES)):
                    best = (RI, RS)
        return best

    RpI, RpS = search(npos)
    RmI, RmS = search(nneg)
    Rw = int(max(1, nwld.max())) if nwld.sum() else 0

    rounds = []
    rounds += [(0, OP_MAX)] * RpI
    rounds += [(0, OP_MIN)] * RmI
    rounds += [(0, OP_FREE)] * Rw
    rounds += [(1, OP_MAX)] * RpS
    rounds += [(2, OP_MIN)] * RmS
    R = len(rounds)

    # order: identity rounds first (their tile lands first), ops mixed
    def _mix(lst):
        by_op = {}
        for q in lst:
            by_op.setdefault(rounds[q][1], []).append(q)
        out = []
        keys = sorted(by_op)
        while any(by_op[kk] for kk in keys):
            for kk in keys:
                if by_op[kk]:
                    out.append(by_op[kk].pop(0))
        return out

    id_rounds = [q for q in range(R) if rounds[q][0] == 0]
    sp_rounds = [q for q in range(R) if rounds[q][0] != 0]
    order = _mix(id_rounds) + _mix(sp_rounds)
    rounds = [rounds[q] for q in order]

    # engine paths: wild -> ACT Square single op ("wild"); of the rest:
    # ~9 ACT-relu rounds, rest DVE; squares all DVE stt.
    paths = []
    n_act = 9
    ai = 0
    tame_rounds = [r for r in range(R) if rounds[r][1] != OP_FREE]
    act_set = set()
    if tame_rounds:
        stride = max(1, len(tame_rounds) / max(n_act, 1))
        act_set = {tame_rounds[min(len(tame_rounds) - 1, int(q * stride))]
                   for q in range(n_act)}
    for r in range(R):
        if rounds[r][1] == OP_FREE:
            paths.append("wild")
        elif r in act_set:
            paths.append("act")
        else:
            paths.append("dve")
    schedule = {"R": R, "rounds": rounds, "paths": paths}

    in_maps = []
    for c in range(NCORES):
        cs = core == c

        def cells_for(n_cf, RI, RS):
            ov = np.maximum(0, n_cf - RI)
            cmap = []
            if RS:
                for feat in np.nonzero(ov)[0]:
                    cmap += [feat] * int(np.ceil(ov[feat] / RS))
            assert len(cmap) <= NF, (c, len(cmap))
            cmap += [0] * (NF - len(cmap))
            return np.array(cmap, np.int64)

        gP = cells_for(npos[c], RpI, RpS)
        gM = cells_for(nneg[c], RmI, RmS)

        s1 = np.zeros((NF, R), np.float32)
        s2 = np.zeros((NF, R), np.float32)
        mval = np.zeros((NF, R), np.float32)
        mjl = np.zeros((NF, R), np.int64)
        used = np.zeros((NF, R), bool)

        r_idx = {key: [q for q in range(R) if rounds[q] == key]
                 for key in set(rounds)}

        def _set(rr, p_, sid):
            assert not used[p_, rr], (c, rr, p_)
            used[p_, rr] = True
            if wild[sid]:
                s1[p_, rr] = P[sid]
                s2[p_, rr] = -Q[sid]
                mval[p_, rr] = 1.0
            else:
                s1[p_, rr] = -t[sid]
                s2[p_, rr] = t[sid]
                mval[p_, rr] = P[sid] * P[sid]
            mjl[p_, rr] = jl[sid]

        def place(slot_ids, id_rounds_, sp_rounds_, gmap):
            order_f = np.argsort(f[slot_ids], kind="stable")
            sids = slot_ids[order_f]
            taken = {}
            spill = []
            for sid in sids:
                feat = f[sid]
                nid = taken.get(feat, 0)
                if nid < len(id_rounds_):
                    _set(id_rounds_[nid], feat, sid)
                    taken[feat] = nid + 1
                else:
                    spill.append(sid)
            if spill:
                cell_of = {}
                for p_, feat in enumerate(gmap):
                    cell_of.setdefault(feat, []).append(p_)
                fill = {}
                for sid in spill:
                    feat = f[sid]
                    cells = cell_of.get(feat)
                    assert cells, (c, feat)
                    n_ = fill.get(feat, 0)
                    ci, ri = n_ % len(cells), n_ // len(cells)
                    assert ri < len(sp_rounds_), (c, feat, n_)
                    _set(sp_rounds_[ri], cells[ci], sid)
                    fill[feat] = n_ + 1

        place(np.nonzero(cs & tame & pos)[0],
              r_idx.get((0, OP_MAX), []), r_idx.get((1, OP_MAX), []), gP)
        place(np.nonzero(cs & tame & neg)[0],
              r_idx.get((0, OP_MIN), []), r_idx.get((2, OP_MIN), []), gM)
        wr = r_idx.get((0, OP_FREE), [])
        wtaken = {}
        for sid in np.nonzero(cs & wild)[0]:
            feat = f[sid]
            n_ = wtaken.get(feat, 0)
            assert n_ < len(wr), (c, feat)
            _set(wr[n_], feat, sid)
            wtaken[feat] = n_ + 1

        XT = np.ascontiguousarray(X.T)
        xt = np.concatenate([XT, XT[gP], XT[gM]], axis=1).astype(_NP16)
        pq = np.concatenate([s1, s2], axis=1).astype(np.float32)
        masks = np.zeros((NF, R, JC), np.float32)
        pp, rr_ = np.nonzero(used)
        masks[pp, rr_, mjl[pp, rr_]] = mval[pp, rr_]
        masks = np.ascontiguousarray(masks.reshape(NF, R * JC)).astype(_NP16)

        in_maps.append({
            "xt": np.ascontiguousarray(xt),
            "pq": np.ascontiguousarray(pq),
            "masks": masks,
        })
    return schedule, in_maps


# ---------------------------------------------------------------- device IR
def _build_program(schedule, legalize=True):
    R = schedule["R"]
    rounds = schedule["rounds"]
    paths = schedule["paths"]

    nc = bass.Bass(enable_asserts=False)
    xt_d = nc.dram_tensor("xt", [NF, 3 * M], _DT16, kind="ExternalInput")
    pq_d = nc.dram_tensor("pq", [NF, 2 * R], _DT, kind="ExternalInput")
    mk_d = nc.dram_tensor("masks", [NF, R * JC], _DT16, kind="ExternalInput")
    e_d = nc.dram_tensor("e_out", [JC, M], _DT, kind="ExternalOutput")

    AF = mybir.ActivationFunctionType
    ALU = mybir.AluOpType
    MK_SPLIT = min(8, R)

    with tile.TileContext(nc) as tc:
        with (
            tc.tile_pool(name="consts", bufs=1) as consts,
            tc.tile_pool(name="up", bufs=4) as up,
            tc.tile_pool(name="r2p", bufs=6) as r2p,
            tc.tile_pool(name="outp", bufs=1) as outp,
            tc.tile_pool(name="psum", bufs=1, space="PSUM") as psum,
        ):
            # ACT table warm-up: no DMA dependency, runs at t~0.
            warm_in = consts.tile([1, 1], _DT)
            nc.gpsimd.memset(warm_in[:], 0.0)
            warm_out = consts.tile([1, 1], _DT)
            nc.scalar.activation(warm_out[:], warm_in[:], AF.Relu)

            pq_sb = consts.tile([NF, 2 * R], _DT)
            nc.scalar.dma_start(pq_sb[:], pq_d[:])
            xt_sb = consts.tile([NF, 3 * M], _DT16)
            nc.scalar.dma_start(xt_sb[:, 0:M], xt_d[:, 0:M])
            mk_sb = consts.tile([NF, R * JC], _DT16)
            nc.scalar.dma_start(
                mk_sb[:, 0:MK_SPLIT * JC], mk_d[:, 0:MK_SPLIT * JC]
            )
            nc.sync.dma_start(xt_sb[:, M:3 * M], xt_d[:, M:3 * M])
            if MK_SPLIT < R:
                nc.sync.dma_start(
                    mk_sb[:, MK_SPLIT * JC:], mk_d[:, MK_SPLIT * JC:]
                )

            e_ps = psum.tile([JC, M], _DT)
            for r in range(R):
                tl, op = rounds[r]
                path = paths[r]
                x_ap = xt_sb[:, tl * M:(tl + 1) * M]
                s1 = pq_sb[:, r:r + 1]
                s2 = pq_sb[:, R + r:R + r + 1]
                r2 = r2p.tile([NF, M], _DT16)
                if path == "wild":
                    # always-on slots: r2 = (P*x - Q)^2, single ACT op
                    nc.scalar.activation(
                        r2[:], x_ap, AF.Square, bias=s2, scale=s1)
                else:
                    u = up.tile([NF, M], _DT16)
                    if path == "dve":
                        if op == OP_MAX:
                            nc.vector.tensor_scalar(
                                u[:], x_ap, s1, 0.0, ALU.add, ALU.max)
                        else:
                            nc.vector.tensor_scalar(
                                u[:], x_ap, s1, 0.0, ALU.add, ALU.min)
                    else:
                        if op == OP_MAX:
                            nc.scalar.activation(u[:], x_ap, AF.Relu, bias=s1)
                        else:
                            nc.scalar.activation(
                                u[:], x_ap, AF.Relu, bias=s2, scale=-1.0)
                    if path == "act" and op == OP_MIN:
                        # u = (t-x)+ ; r2 = u^2 = (x-t)*(-u) ... sign folded:
                        # (x + (-t)) * u = -(u^2); fix via negated mask? No:
                        # use stt with min-u: (x-t)*min(x-t,0) = u^2 when
                        # u=min(...); for ACT-min u=relu(t-x) >= 0, and
                        # (x-t)*u = -u^2.  So square via stt on (t - x):
                        # instead just multiply u*u.
                        nc.vector.tensor_tensor(r2[:], u[:], u[:], ALU.mult)
                    else:
                        # r2 = (x + (-t)) * u  == relu(x-t)^2 (max rounds)
                        # or min(x-t,0)^2 (dve min rounds: u<=0, product>=0)
                        nc.vector.scalar_tensor_tensor(
                            r2[:], x_ap, s1, u[:], ALU.add, ALU.mult)
                nc.tensor.matmul(
                    e_ps[:], mk_sb[:, r * JC:(r + 1) * JC], r2[:],
                    start=(r == 0), stop=(r == R - 1),
                )

            e_sb = outp.tile([JC, M], _DT)
            nc.scalar.copy(e_sb[:], e_ps[:])
            nc.sync.dma_start(e_d[:], e_sb[:])
    if legalize:
        _legalize_waits(nc)
    return nc


# ---------------------------------------------------------------- profiling
def _install_ntff_shim():
    import sys
    import types

    if "antenv.axon_hooks" in sys.modules:
        return
    from trn_agent_boot.trn_boot import _ntff_profile_via_ctypes

    hook = _ntff_profile_via_ctypes("/opt/axon/libaxon_pjrt.so")
    mod = types.ModuleType("antenv.axon_hooks")
    mod.get_axon_ntff_profile_hook = lambda: hook
    mod.set_axon_ntff_profile_hook = lambda h: None
    sys.modules["antenv.axon_hooks"] = mod


# ---------------------------------------------------------------- entrypoint
def kernel(X, A_vals, V, W, Fvec, A_rows, A_cols, _want_trace=False):
    if _want_trace:
        _install_ntff_shim()
    schedule, in_maps = _prepare(X, A_vals, V, W, Fvec, A_rows, A_cols)
    nc = _build_program(schedule)
    res = run_bass_kernel_spmd(
        nc, in_maps, core_ids=list(range(NCORES)), trace=_want_trace
    )
    F = np.asarray(Fvec, dtype=np.float32)
    H = np.zeros(M, dtype=np.float32)
    for c in range(NCORES):
        E_c = res.results[c]["e_out"]
        H += F[c * JC:(c + 1) * JC] @ np.exp(-E_c)
    kernel.last_result = res
    return H.astype(np.float32)


# revision 11
# speedup vs baseline: 1.2717x; 1.2717x over previous
"""Trainium2 Bass kernel for nn_DFE_model (gnn_message_passing).

Math: reference scatters upd[m,i] = A_vals[i]*X[m, A_cols[i]//2] -
V[A_rows[i], A_cols[i]] into D[m, :, :] (last write wins per (row, col)),
then H[m] = sum_j F[j] * exp(-sum_k W[j,k]*relu(D[m,j,k])^2).

Per active slot s (j, k, f=k//2) with P = sqrt(W)*a, Q = sqrt(W)*V, the
contribution to E[j, m] is relu(P*x[m,f] - Q)^2.  Layout: X^T stays
resident in SBUF as three [128, 512] feature tiles (identity + one spill
tile per sign); each batch of 128 slots (a "round") is one per-partition
affine+relu over a resident tile followed by a relu^2 product and a
[128 slot -> 64 j] fp16 mask matmul accumulating E[64, 512] in PSUM.
P^2 is folded into the mask weights, so the per-round ops are:
  u  = max(x + (-t), 0)   (P>0 rounds; min for P<0 -- square kills sign)
  r2 = (x + (-t)) * u     (scalar_tensor_tensor == relu(x-t)^2 exactly)
both InstTensorScalarPtr (fast DVE mode), or ACT Relu with per-partition
scale/bias for rounds assigned to the scalar engine.

Host classifies slots exactly (per-feature min/max of the actual X):
sure-zero dropped; wild always-on slots (|t| > 100 or P == 0) use one
ACT Square(scale=P, bias=-Q) round (no relu needed, mask 1.0); the rest
pack into sign-homogeneous rounds on 3 resident tiles.  E is copied
PSUM->SBUF and DMAed out per core; the host does exp and the F-weighted
reduction (cheap: 8 x [64, 512]).
"""

import numpy as np

import concourse.bass as bass
import concourse.mybir as mybir
import concourse.tile as tile
from concourse.bass_utils import run_bass_kernel_spmd

# ---------------------------------------------------------------- constants
M = 512
J = 512
K = 256
NF = 128
NCORES = 8
JC = J // NCORES

_DT = mybir.dt.float32
_DT16 = mybir.dt.float16
_NP16 = np.float16

WILD_T = 100.0

OP_MAX, OP_MIN, OP_FREE = 0, 1, 2


# ------------------------------------------------------- walrus wait limit
def _legalize_waits(nc, max_waits=1):
    n = 0
    for f in nc.m.functions:
        for b in f.blocks:
            out, changed = [], False
            for inst in list(b.instructions):
                si = inst.sync_info
                waits = list(si.on_wait) if si and si.on_wait else []
                if len(waits) > max_waits:
                    for w in waits[max_waits:]:
                        n += 1
                        nop = mybir.InstNoOp(name=f"waitfix_{n}", ins=[], outs=[])
                        nop.engine = inst.engine
                        nop.sync_info = mybir.SyncInfo(on_wait=[w], on_update=[])
                        out.append(nop)
                    si.on_wait = waits[:max_waits]
                    changed = True
                out.append(inst)
            if changed:
                b.instructions = out


# ------------------------------------------------ slim Tile exit barrier
def _slim_drain_and_barrier(self, tick_clock, wait_clock):
    from concourse.vector_clock import ScopedClock

    drain_sp = self.nc.sync.drain()
    wait_clock.add_sem_waits(
        drain_sp.ins, ScopedClock({None: tick_clock.global_clock})
    )
    drain_gp = self.nc.gpsimd.drain()
    wait_clock.add_sem_waits(
        drain_gp.ins, ScopedClock({None: tick_clock.global_clock})
    )
    assert self.sems is not None
    popped = self.nc._tile_sem_poison_stack.pop()
    assert popped is self._sem_poison
    self.nc.clear_and_free_semaphores(list(self.sems.allocated().values()))


tile.TileContext._drain_and_barrier = _slim_drain_and_barrier


# ---------------------------------------------------------------- packing
def _prepare(X, A_vals, V, W, Fvec, A_rows, A_cols):
    rows = np.asarray(A_rows).astype(np.int64)
    cols = np.asarray(A_cols).astype(np.int64)
    X = np.asarray(X, dtype=np.float32)
    A_vals = np.asarray(A_vals, dtype=np.float32)
    V = np.asarray(V, dtype=np.float32)
    W = np.asarray(W, dtype=np.float32)
    Fvec = np.asarray(Fvec, dtype=np.float32)

    nnz = rows.shape[0]
    lin = rows * K + cols
    winner = np.full(J * K, -1, dtype=np.int64)
    winner[lin] = np.arange(nnz)
    active = np.nonzero(winner >= 0)[0]
    i = winner[active]
    j = active // K
    k = active % K
    f = k // 2
    s = np.sqrt(W[j, k]).astype(np.float32)
    P = s * A_vals[i]
    Q = s * V[j, k]

    xmin = X.min(axis=0)
    xmax = X.max(axis=0)
    with np.errstate(divide="ignore", invalid="ignore"):
        t = np.where(P != 0, Q / np.where(P == 0, 1.0, P), 0.0)
    pos = P > 0
    neg = P < 0
    zer = P == 0
    sure_zero = (
        (pos & (t >= xmax[f])) | (neg & (t <= xmin[f])) | (zer & (Q >= 0))
    )
    sure_on = (
        (pos & (t <= xmin[f])) | (neg & (t >= xmax[f])) | (zer & (Q < 0))
    )
    keep = ~sure_zero
    wild = keep & sure_on & (zer | (np.abs(t) > WILD_T))
    tame = keep & ~wild

    core = j // JC
    jl = j % JC

    npos = np.zeros((NCORES, NF), np.int64)
    nneg = np.zeros((NCORES, NF), np.int64)
    nwld = np.zeros((NCORES, NF), np.int64)
    for c in range(NCORES):
        cs = core == c
        npos[c] = np.bincount(f[cs & tame & pos], minlength=NF)
        nneg[c] = np.bincount(f[cs & tame & neg], minlength=NF)
        nwld[c] = np.bincount(f[cs & wild], minlength=NF)

    def spill_ok(n_cf, RI, RS):
        ov = np.maximum(0, n_cf - RI)
        if RS == 0:
            return not np.any(ov > 0)
        return np.ceil(ov / RS).sum() <= NF

    def search(n_all):
        best = None
        for RI in range(0, 20):
            for RS in range(0, 10):
                if best is not None and RI + RS >= best[0] + best[1]:
                    continue
                if all(spill_ok(n_all[c], RI, RS) for c in range(NCORES)):
                    best = (RI, RS)
        return best

    RpI, RpS = search(npos)
    RmI, RmS = search(nneg)
    Rw = int(max(1, nwld.max())) if nwld.sum() else 0

    rounds = []
    rounds += [(0, OP_MAX)] * RpI
    rounds += [(0, OP_MIN)] * RmI
    rounds += [(0, OP_FREE)] * Rw
    rounds += [(1, OP_MAX)] * RpS
    rounds += [(2, OP_MIN)] * RmS
    R = len(rounds)

    # order: identity rounds first (their tile lands first), ops mixed
    def _mix(lst):
        by_op = {}
        for q in lst:
            by_op.setdefault(rounds[q][1], []).append(q)
        out = []
        keys = sorted(by_op)
        while any(by_op[kk] for kk in keys):
            for kk in keys:
                if by_op[kk]:
                    out.append(by_op[kk].pop(0))
        return out

    id_rounds = [q for q in range(R) if rounds[q][0] == 0]
    sp_rounds = [q for q in range(R) if rounds[q][0] != 0]
    order = _mix(id_rounds) + _mix(sp_rounds)
    rounds = [rounds[q] for q in order]

    # engine paths: wild -> ACT Square single op ("wild").  Of the tame
    # rounds: min-rounds prefer ACT (their u >= 0 squares via fast ts-pow),
    # max-rounds prefer DVE; target ~9-10 ACT rounds, alternating with DVE
    # rounds so neither engine sees long runs.
    tame_rounds = [r for r in range(R) if rounds[r][1] != OP_FREE]
    n_act = min(9, len(tame_rounds))
    mins = [r for r in tame_rounds if rounds[r][1] == OP_MIN]
    maxs = [r for r in tame_rounds if rounds[r][1] == OP_MAX]
    act_list = mins[:n_act]
    if len(act_list) < n_act:
        act_list += maxs[:n_act - len(act_list)]
    act_set = set(act_list)
    paths = []
    for r in range(R):
        if rounds[r][1] == OP_FREE:
            paths.append("wild")
        elif r in act_set:
            paths.append("act")
        else:
            paths.append("dve")
    schedule = {"R": R, "rounds": rounds, "paths": paths}

    in_maps = []
    for c in range(NCORES):
        cs = core == c

        def cells_for(n_cf, RI, RS):
            ov = np.maximum(0, n_cf - RI)
            cmap = []
            if RS:
                for feat in np.nonzero(ov)[0]:
                    cmap += [feat] * int(np.ceil(ov[feat] / RS))
            assert len(cmap) <= NF, (c, len(cmap))
            cmap += [0] * (NF - len(cmap))
            return np.array(cmap, np.int64)

        gP = cells_for(npos[c], RpI, RpS)
        gM = cells_for(nneg[c], RmI, RmS)

        s1 = np.zeros((NF, R), np.float32)
        s2 = np.zeros((NF, R), np.float32)
        mval = np.zeros((NF, R), np.float32)
        mjl = np.zeros((NF, R), np.int64)
        used = np.zeros((NF, R), bool)

        r_idx = {key: [q for q in range(R) if rounds[q] == key]
                 for key in set(rounds)}

        def _set(rr, p_, sid):
            assert not used[p_, rr], (c, rr, p_)
            used[p_, rr] = True
            if wild[sid]:
                s1[p_, rr] = P[sid]
                s2[p_, rr] = -Q[sid]
                mval[p_, rr] = 1.0
            else:
                s1[p_, rr] = -t[sid]
                s2[p_, rr] = t[sid]
                mval[p_, rr] = P[sid] * P[sid]
            mjl[p_, rr] = jl[sid]

        def place(slot_ids, id_rounds_, sp_rounds_, gmap):
            order_f = np.argsort(f[slot_ids], kind="stable")
            sids = slot_ids[order_f]
            taken = {}
            spill = []
            for sid in sids:
                feat = f[sid]
                nid = taken.get(feat, 0)
                if nid < len(id_rounds_):
                    _set(id_rounds_[nid], feat, sid)
                    taken[feat] = nid + 1
                else:
                    spill.append(sid)
            if spill:
                cell_of = {}
                for p_, feat in enumerate(gmap):
                    cell_of.setdefault(feat, []).append(p_)
                fill = {}
                for sid in spill:
                    feat = f[sid]
                    cells = cell_of.get(feat)
                    assert cells, (c, feat)
                    n_ = fill.get(feat, 0)
                    ci, ri = n_ % len(cells), n_ // len(cells)
                    assert ri < len(sp_rounds_), (c, feat, n_)
                    _set(sp_rounds_[ri], cells[ci], sid)
                    fill[feat] = n_ + 1

        place(np.nonzero(cs & tame & pos)[0],
              r_idx.get((0, OP_MAX), []), r_idx.get((1, OP_MAX), []), gP)
        place(np.nonzero(cs & tame & neg)[0],
              r_idx.get((0, OP_MIN), []), r_idx.get((2, OP_MIN), []), gM)
        wr = r_idx.get((0, OP_FREE), [])
        wtaken = {}
        for sid in np.nonzero(cs & wild)[0]:
            feat = f[sid]
            n_ = wtaken.get(feat, 0)
            assert n_ < len(wr), (c, feat)
            _set(wr[n_], feat, sid)
            wtaken[feat] = n_ + 1

        XT = np.ascontiguousarray(X.T)
        xt = np.concatenate([XT, XT[gP], XT[gM]], axis=1).astype(_NP16)
        pq = np.concatenate([s1, s2], axis=1).astype(np.float32)
        masks = np.zeros((NF, R, JC), np.float32)
        pp, rr_ = np.nonzero(used)
        masks[pp, rr_, mjl[pp, rr_]] = mval[pp, rr_]
        masks = np.ascontiguousarray(masks.reshape(NF, R * JC)).astype(_NP16)

        in_maps.append({
            "xt": np.ascontiguousarray(xt),
            "pq": np.ascontiguousarray(pq),
            "masks": masks,
        })
    return schedule, in_maps


# ---------------------------------------------------------------- device IR
def _build_program(schedule, legalize=True):
    R = schedule["R"]
    rounds = schedule["rounds"]
    paths = schedule["paths"]

    nc = bass.Bass(enable_asserts=False)
    xt_d = nc.dram_tensor("xt", [NF, 3 * M], _DT16, kind="ExternalInput")
    pq_d = nc.dram_tensor("pq", [NF, 2 * R], _DT, kind="ExternalInput")
    mk_d = nc.dram_tensor("masks", [NF, R * JC], _DT16, kind="ExternalInput")
    e_d = nc.dram_tensor("e_out", [JC, M], _DT16, kind="ExternalOutput")

    AF = mybir.ActivationFunctionType
    ALU = mybir.AluOpType
    MK_SPLIT = min(4, R)

    with tile.TileContext(nc) as tc:
        with (
            tc.tile_pool(name="consts", bufs=1) as consts,
            tc.tile_pool(name="up", bufs=8) as up,
            tc.tile_pool(name="r2p", bufs=10) as r2p,
            tc.tile_pool(name="outp", bufs=1) as outp,
            tc.tile_pool(name="psum", bufs=1, space="PSUM") as psum,
        ):
            # ACT table warm-up: no DMA dependency, runs at t~0.
            with tc.high_priority():
                warm_in = consts.tile([1, 1], _DT)
                nc.vector.memset(warm_in[:], 0.0)
                warm_out = consts.tile([1, 1], _DT)
                nc.scalar.activation(warm_out[:], warm_in[:], AF.Relu)

            pq_sb = consts.tile([NF, 2 * R], _DT)
            nc.scalar.dma_start(pq_sb[:], pq_d[:])
            xt_sb = consts.tile([NF, 3 * M], _DT16)
            nc.sync.dma_start(xt_sb[:, 0:M], xt_d[:, 0:M])
            mk_sb = consts.tile([NF, R * JC], _DT16)
            nc.scalar.dma_start(
                mk_sb[:, 0:MK_SPLIT * JC], mk_d[:, 0:MK_SPLIT * JC]
            )
            if MK_SPLIT < R:
                nc.sync.dma_start(
                    mk_sb[:, MK_SPLIT * JC:], mk_d[:, MK_SPLIT * JC:]
                )
            nc.sync.dma_start(xt_sb[:, M:3 * M], xt_d[:, M:3 * M])

            e_ps = psum.tile([JC, M], _DT)
            for r in range(R):
                tl, op = rounds[r]
                path = paths[r]
                x_ap = xt_sb[:, tl * M:(tl + 1) * M]
                s1 = pq_sb[:, r:r + 1]
                s2 = pq_sb[:, R + r:R + r + 1]
                r2 = r2p.tile([NF, M], _DT16)
                if path == "wild":
                    # always-on slots: r2 = (P*x - Q)^2, single ACT op
                    nc.scalar.activation(
                        r2[:], x_ap, AF.Square, bias=s2, scale=s1)
                else:
                    u = up.tile([NF, M], _DT16)
                    if path == "dve":
                        if op == OP_MAX:
                            nc.vector.tensor_scalar(
                                u[:], x_ap, s1, 0.0, ALU.add, ALU.max)
                        else:
                            nc.vector.tensor_scalar(
                                u[:], x_ap, s1, 0.0, ALU.add, ALU.min)
                    else:
                        if op == OP_MAX:
                            nc.scalar.activation(u[:], x_ap, AF.Relu, bias=s1)
                        else:
                            nc.scalar.activation(
                                u[:], x_ap, AF.Relu, bias=s2, scale=-1.0)
                    nc.vector.tensor_tensor(r2[:], u[:], u[:], ALU.mult)
                nc.tensor.matmul(
                    e_ps[:], mk_sb[:, r * JC:(r + 1) * JC], r2[:],
                    start=(r == 0), stop=(r == R - 1),
                )

            e_sb = outp.tile([JC, M], _DT16)
            nc.scalar.copy(e_sb[:], e_ps[:])
            nc.sync.dma_start(e_d[:], e_sb[:])
    if legalize:
        _legalize_waits(nc)
    return nc


# ---------------------------------------------------------------- profiling
def _install_ntff_shim():
    import sys
    import types

    if "antenv.axon_hooks" in sys.modules:
        return
    from trn_agent_boot.trn_boot import _ntff_profile_via_ctypes

    hook = _ntff_profile_via_ctypes("/opt/axon/libaxon_pjrt.so")
    mod = types.ModuleType("antenv.axon_hooks")
    mod.get_axon_ntff_profile_hook = lambda: hook
    mod.set_axon_ntff_profile_hook = lambda h: None
    sys.modules["antenv.axon_hooks"] = mod


# ---------------------------------------------------------------- entrypoint
def kernel(X, A_vals, V, W, Fvec, A_rows, A_cols, _want_trace=False):
    if _want_trace:
        _install_ntff_shim()
    schedule, in_maps = _prepare(X, A_vals, V, W, Fvec, A_rows, A_cols)
    nc = _build_program(schedule)
    res = run_bass_kernel_spmd(
        nc, in_maps, core_ids=list(range(NCORES)), trace=_want_trace
    )
    F = np.asarray(Fvec, dtype=np.float32)
    H = np.zeros(M, dtype=np.float32)
    for c in range(NCORES):
        E_c = res.results[c]["e_out"].astype(np.float32)
        H += F[c * JC:(c + 1) * JC] @ np.exp(-E_c)
    kernel.last_result = res
    return H.astype(np.float32)
